# revision 6
# baseline (speedup 1.0000x reference)
"""Trainium2 Bass kernel for NeuralODE (encoder -> RK4x20 of MLP dynamics -> decoder).

Data-parallel over 8 NeuronCores: batch 65536 -> 8 x 8192 columns.
Feature-major layout on device: activations are [features(partitions), batch(free)].
Matmuls run in float32r (fp22 truncation, 1 cycle/row). RK4 combine uses the
"stored h = true h - dt*b3" trick so that per f-eval only 2 DVE ops are needed;
all b3 contributions are folded into the next layer-1 tanh bias.
"""

import sys

import numpy as np

_REPO = "/opt/trn_rl_repo"
if _REPO not in sys.path:
    sys.path.insert(0, _REPO)

from contextlib import ExitStack  # noqa: E402

import concourse.bass as bass  # noqa: E402
import concourse.tile as tile  # noqa: E402
from concourse import bacc, mybir  # noqa: E402
from concourse.bass_utils import run_bass_kernel_spmd  # noqa: E402

F32 = mybir.dt.float32
F32R = mybir.dt.float32r
TANH = mybir.ActivationFunctionType.Tanh
IDENT = mybir.ActivationFunctionType.Identity
MULT = mybir.AluOpType.mult
ADD = mybir.AluOpType.add
BYPASS = mybir.AluOpType.bypass

DT = 0.05  # T_SPAN / N_STEPS
N_CORES = 8
B_FULL = 65536
D_IN = 64
H = 128
H2 = 256
D_OUT = 64

# bias column indices in the packed [128, 12] bias tensor
BC_ENC = 0
BC_L1 = {0: (1, 2), 1: (3, 4), 2: (3, 4), 3: (5, 6)}  # per-eval L1 tanh bias cols
BC_B2 = (7, 8)
BC_6B3 = 9
BC_MDTB3 = 10
BC_DEC = 11


def _emit(nc, B, n_steps, G=1024):
    """Emit the per-core tile program. B columns, G columns per group."""
    ng = B // G
    assert B % G == 0 and G % 512 == 0 and ng % 2 == 0
    ns = G // 512  # 512-col MM slices per group

    xT_d = nc.dram_tensor("xT", [D_IN, B], F32, kind="ExternalInput").ap()
    wenc_d = nc.dram_tensor("W_enc", [D_IN, H], F32, kind="ExternalInput").ap()
    w1_d = nc.dram_tensor("W1", [H, H2], F32, kind="ExternalInput").ap()
    w2a_d = nc.dram_tensor("W2a", [H, H2], F32, kind="ExternalInput").ap()
    w2b_d = nc.dram_tensor("W2b", [H, H2], F32, kind="ExternalInput").ap()
    w3a_d = nc.dram_tensor("W3a", [H, H], F32, kind="ExternalInput").ap()
    w3b_d = nc.dram_tensor("W3b", [H, H], F32, kind="ExternalInput").ap()
    wdec_d = nc.dram_tensor("Wdec", [H, D_OUT], F32, kind="ExternalInput").ap()
    bias_d = nc.dram_tensor("biases", [H, 12], F32, kind="ExternalInput").ap()
    yT_d = nc.dram_tensor("yT", [D_OUT, B], F32, kind="ExternalOutput").ap()

    with tile.TileContext(nc) as tc, ExitStack() as ctx:
        wp = ctx.enter_context(tc.tile_pool(name="weights", bufs=1))
        wenc = wp.tile([D_IN, H], F32, name="wenc")
        w1f = wp.tile([H, H2], F32, name="w1f")
        w2af = wp.tile([H, H2], F32, name="w2af")
        w2bf = wp.tile([H, H2], F32, name="w2bf")
        w3af = wp.tile([H, H], F32, name="w3af")
        w3bf = wp.tile([H, H], F32, name="w3bf")
        wdecf = wp.tile([H, D_OUT], F32, name="wdecf")
        bs = wp.tile([H, 12], F32, name="bs")
        for sb, d in ((wenc, wenc_d), (w1f, w1_d), (w2af, w2a_d), (w2bf, w2b_d),
                      (w3af, w3a_d), (w3bf, w3b_d), (wdecf, wdec_d), (bs, bias_d)):
            nc.gpsimd.dma_start(sb[:], d[:])
        # f32r matmul operands must be produced pre-rounded: ACT-convert once
        w1 = wp.tile([H, H2], F32R, name="w1")
        w2a = wp.tile([H, H2], F32R, name="w2a")
        w2b = wp.tile([H, H2], F32R, name="w2b")
        w3a = wp.tile([H, H], F32R, name="w3a")
        w3b = wp.tile([H, H], F32R, name="w3b")
        wdec = wp.tile([H, D_OUT], F32R, name="wdec")
        for dst, src in ((w1, w1f), (w2a, w2af), (w2b, w2bf), (w3a, w3af),
                         (w3b, w3bf), (wdec, wdecf)):
            nc.scalar.activation(dst[:], src[:], IDENT)

        # persistent h tiles: unique names (one tag each), 1 buf per tag
        hp = ctx.enter_context(tc.tile_pool(name="h", bufs=1))
        h_t = [[hp.tile([H, G], F32R, name=f"h{g}_{p}") for p in range(2)]
               for g in range(ng)]

        # rotating pools: ONE shared tag per pool so bufs actually rotate
        zp = ctx.enter_context(tc.tile_pool(name="zpsum", bufs=2, space="PSUM"))
        z1p = ctx.enter_context(tc.tile_pool(name="z1sb", bufs=2))
        z2p = ctx.enter_context(tc.tile_pool(name="z2sb", bufs=2))
        up = ctx.enter_context(tc.tile_pool(name="upool", bufs=4))
        ap_ = ctx.enter_context(tc.tile_pool(name="apool", bufs=4))
        xp = ctx.enter_context(tc.tile_pool(name="xin", bufs=2))
        yp = ctx.enter_context(tc.tile_pool(name="yout", bufs=2))

        def ztile():
            return zp.tile([H, 2 * G], F32, name="z", tag="z")

        def mm(out, lhsT, rhs, start, stop):
            nc.tensor.matmul(out, lhsT, rhs, start=start, stop=stop)

        def col(c, p=H):
            return bs[0:p, c:c + 1]

        # ---- encoder: h0_stored = tanh(x @ W_enc + b_enc) - dt*b3 ----
        for g in range(ng):
            xsb = xp.tile([D_IN, G], F32, name="x", tag="x")
            nc.gpsimd.dma_start(xsb[:], xT_d[:, g * G:(g + 1) * G])
            e = ztile()
            for s in range(ns):
                sl = slice(s * 512, (s + 1) * 512)
                mm(e[:, sl], wenc[:], xsb[:, sl], True, True)
            tmp = up.tile([H, G], F32R, name="u", tag="u")
            nc.scalar.activation(tmp[:], e[:, 0:G], TANH, bias=col(BC_ENC))
            nc.vector.tensor_scalar(h_t[g][0][:], tmp[:], col(BC_MDTB3), None, ADD)

        # ---- RK4 steps ----
        for step in range(n_steps):
            par, nxt = step % 2, (step + 1) % 2
            for pair in range(ng // 2):
                gs = (2 * pair, 2 * pair + 1)
                u = [h_t[g][par] for g in gs]
                acc = [None, None]
                for ev in range(4):
                    ba, bb = BC_L1[ev]
                    z1sb, z2sb, kps = [None, None], [None, None], [None, None]
                    # L1 matmuls
                    z1ps = [None, None]
                    for i, g in enumerate(gs):
                        z1 = ztile()
                        z1ps[i] = z1
                        for m in range(2):
                            lw = w1[:, m * H:(m + 1) * H]
                            for s in range(ns):
                                sl = slice(s * 512, (s + 1) * 512)
                                osl = slice(m * G + s * 512, m * G + (s + 1) * 512)
                                mm(z1[:, osl], lw, u[i][:, sl], True, True)
                    # tanh(z1 + b1_eff)
                    for i, g in enumerate(gs):
                        t = z1p.tile([H, 2 * G], F32R, name="z1s", tag="z1s")
                        z1sb[i] = t
                        nc.scalar.activation(t[:, 0:G], z1ps[i][:, 0:G], TANH,
                                             bias=col(ba))
                        nc.scalar.activation(t[:, G:2 * G], z1ps[i][:, G:2 * G],
                                             TANH, bias=col(bb))
                    # L2 matmuls (accumulate over the two z1 feature chunks)
                    z2ps = [None, None]
                    for i, g in enumerate(gs):
                        z2 = ztile()
                        z2ps[i] = z2
                        for m in range(2):
                            for kc, (wt, st, sp) in enumerate(
                                    ((w2a, True, False), (w2b, False, True))):
                                lw = wt[:, m * H:(m + 1) * H]
                                for s in range(ns):
                                    rsl = slice(kc * G + s * 512,
                                                kc * G + (s + 1) * 512)
                                    osl = slice(m * G + s * 512,
                                                m * G + (s + 1) * 512)
                                    mm(z2[:, osl], lw, z1sb[i][:, rsl], st, sp)
                    # tanh(z2 + b2)
                    for i, g in enumerate(gs):
                        t = z2p.tile([H, 2 * G], F32R, name="z2s", tag="z2s")
                        z2sb[i] = t
                        nc.scalar.activation(t[:, 0:G], z2ps[i][:, 0:G], TANH,
                                             bias=col(BC_B2[0]))
                        nc.scalar.activation(t[:, G:2 * G], z2ps[i][:, G:2 * G],
                                             TANH, bias=col(BC_B2[1]))
                    # L3 matmuls -> K_raw (no b3)
                    for i, g in enumerate(gs):
                        k = ztile()
                        kps[i] = k
                        for kc, (wt, st, sp) in enumerate(
                                ((w3a, True, False), (w3b, False, True))):
                            for s in range(ns):
                                rsl = slice(kc * G + s * 512, kc * G + (s + 1) * 512)
                                sl = slice(s * 512, (s + 1) * 512)
                                mm(k[:, sl], wt[:], z2sb[i][:, rsl], st, sp)
                    # DVE: u_{next} = c*K_raw + h_stored, acc updates, combine
                    if ev < 3:
                        c = DT / 2 if ev < 2 else DT
                        un = [None, None]
                        for i, g in enumerate(gs):
                            t = up.tile([H, G], F32R, name="u", tag="u")
                            un[i] = t
                            nc.vector.scalar_tensor_tensor(
                                t[:], kps[i][:, 0:G], c, h_t[g][par][:],
                                MULT, ADD)
                        for i, g in enumerate(gs):
                            a = ap_.tile([H, G], F32, name="a", tag="a")
                            if ev == 0:
                                nc.vector.tensor_scalar(
                                    a[:], kps[i][:, 0:G], col(BC_6B3), None, ADD)
                            else:
                                nc.vector.scalar_tensor_tensor(
                                    a[:], kps[i][:, 0:G], 2.0, acc[i][:],
                                    MULT, ADD)
                            acc[i] = a
                        u = un
                    else:
                        for i, g in enumerate(gs):
                            t = up.tile([H, G], F32R, name="u", tag="u")
                            nc.vector.scalar_tensor_tensor(
                                t[:], kps[i][:, 0:G], 0.0, acc[i][:], BYPASS, ADD)
                            nc.vector.scalar_tensor_tensor(
                                h_t[g][nxt][:], t[:], DT / 6, h_t[g][par][:],
                                MULT, ADD)

        # ---- decoder: y = h_stored @ Wdec + b_dec_eff ----
        fin = n_steps % 2
        for g in range(ng):
            yps = ztile()
            for s in range(ns):
                sl = slice(s * 512, (s + 1) * 512)
                mm(yps[0:D_OUT, sl], wdec[:], h_t[g][fin][:, sl], True, True)
            ysb = yp.tile([D_OUT, G], F32, name="y", tag="y")
            nc.scalar.activation(ysb[:], yps[0:D_OUT, 0:G], IDENT,
                                 bias=col(BC_DEC, D_OUT))
            nc.gpsimd.dma_start(yT_d[:, g * G:(g + 1) * G], ysb[:])


_CACHE = {}


def build_module(B=B_FULL // N_CORES, n_steps=20, G=1024):
    key = (B, n_steps, G)
    if key not in _CACHE:
        nc = bacc.Bacc("TRN2", target_bir_lowering=False, debug=False,
                       num_devices=N_CORES)
        _emit(nc, B, n_steps, G)
        nc.compile()
        _CACHE[key] = nc
    return _CACHE[key]


def make_in_maps(inputs, n_cores=N_CORES, B=B_FULL):
    x = np.asarray(inputs["x"], np.float32)
    W_enc = np.asarray(inputs["W_enc"], np.float32)
    b_enc = np.asarray(inputs["b_enc"], np.float32)
    W1 = np.asarray(inputs["W1"], np.float32)
    b1 = np.asarray(inputs["b1"], np.float32)
    W2 = np.asarray(inputs["W2"], np.float32)
    b2 = np.asarray(inputs["b2"], np.float32)
    W3 = np.asarray(inputs["W3"], np.float32)
    b3 = np.asarray(inputs["b3"], np.float32)
    W_dec = np.asarray(inputs["W_dec"], np.float32)
    b_dec = np.asarray(inputs["b_dec"], np.float32)

    b3W1 = b3 @ W1  # [256]
    biases = np.zeros((H, 12), np.float32)
    biases[:, BC_ENC] = b_enc
    for cols, f in ((BC_L1[0], 1.0), (BC_L1[1], 1.5), (BC_L1[3], 2.0)):
        v = b1 + f * DT * b3W1
        biases[:, cols[0]] = v[:H]
        biases[:, cols[1]] = v[H:]
    biases[:, BC_B2[0]] = b2[:H]
    biases[:, BC_B2[1]] = b2[H:]
    biases[:, BC_6B3] = 6.0 * b3
    biases[:, BC_MDTB3] = -DT * b3
    biases[:D_OUT, BC_DEC] = b_dec + DT * (b3 @ W_dec)

    xT = np.ascontiguousarray(x.T)  # [64, B]
    bc = B // n_cores
    common = {
        "W_enc": W_enc, "W1": W1,
        "W2a": np.ascontiguousarray(W2[:H]), "W2b": np.ascontiguousarray(W2[H:]),
        "W3a": np.ascontiguousarray(W3[:H]), "W3b": np.ascontiguousarray(W3[H:]),
        "Wdec": W_dec, "biases": biases,
    }
    return [
        {"xT": np.ascontiguousarray(xT[:, i * bc:(i + 1) * bc]), **common}
        for i in range(n_cores)
    ]


def run(inputs, trace=False):
    nc = build_module()
    in_maps = make_in_maps(inputs)
    br = run_bass_kernel_spmd(nc, in_maps, core_ids=list(range(N_CORES)),
                              trace=trace)
    yT = np.concatenate([br.results[i]["yT"] for i in range(N_CORES)], axis=1)
    y = np.ascontiguousarray(yT.T).astype(np.float32)
    return y, br


def kernel(**inputs):
    y, _ = run(inputs, trace=False)
    return y
